# revision 2
# baseline (speedup 1.0000x reference)
import sys
import numpy as np

sys.path.insert(0, "/opt/pypackages")
sys.path.insert(0, "/opt/trn_rl_repo")

NEG = np.float32(-1e9)
NT, W = 82, 110                  # 82 tempi (28..109), blocks right-aligned in 110 cols
INTERVALS = np.arange(28, 110)
FIRST_IDX = np.cumsum(INTERVALS) - INTERVALS
LAST_IDX = np.cumsum(INTERVALS) - 1
S = int(INTERVALS.sum())         # 5617
ROWMAP = np.concatenate([np.full(t, i, np.int64) for i, t in enumerate(INTERVALS)])
COLMAP = np.concatenate([np.arange(W - t, W) for t in INTERVALS])
POS = np.concatenate([np.arange(t) for t in INTERVALS])


def _trans_log():
    ratio = INTERVALS[None, :].astype(np.float64) / INTERVALS[:, None]
    raw = -100.0 * np.abs(ratio - 1.0)
    mx = raw.max(1, keepdims=True)
    t = raw - np.log(np.exp(raw - mx).sum(1, keepdims=True)) - mx
    return t.astype(np.float32)


def _log_sigmoid(x):
    x = x.astype(np.float32)
    with np.errstate(over="ignore"):
        out = np.where(x >= 0, -np.log1p(np.exp(-x)), x - np.log1p(np.exp(x)))
    return out.astype(np.float32)


_NC_CACHE = {}


def _build(steps):
    if steps in _NC_CACHE:
        return _NC_CACHE[steps]
    from contextlib import ExitStack
    import concourse.bass as bass
    from concourse import mybir

    f32 = mybir.dt.float32
    ADD = mybir.AluOpType.add
    IDENT = mybir.ActivationFunctionType.Identity
    AX = mybir.AxisListType.X

    nc = bass.Bass()
    trans_d = nc.dram_tensor("trans", [NT, NT], f32, kind="ExternalInput")
    mask_d = nc.dram_tensor("mask", [NT, W], f32, kind="ExternalInput")
    ident_d = nc.dram_tensor("ident", [NT, NT], f32, kind="ExternalInput")
    d0_d = nc.dram_tensor("delta0", [NT, W], f32, kind="ExternalInput")
    b_d = nc.dram_tensor("beat", [NT, steps], f32, kind="ExternalInput")
    nb_d = nc.dram_tensor("nonbeat", [NT, steps], f32, kind="ExternalInput")
    last_o = nc.dram_tensor("lastout", [NT, steps], f32, kind="ExternalOutput")
    df_o = nc.dram_tensor("deltaout", [NT, W], f32, kind="ExternalOutput")

    ctx = ExitStack()
    with ctx:
        trans = ctx.enter_context(nc.sbuf_tensor("t_trans", [NT, NT], f32))
        mask = ctx.enter_context(nc.sbuf_tensor("t_mask", [NT, W], f32))
        ident = ctx.enter_context(nc.sbuf_tensor("t_ident", [NT, NT], f32))
        bsb = ctx.enter_context(nc.sbuf_tensor("t_bsb", [NT, steps], f32))
        nbsb = ctx.enter_context(nc.sbuf_tensor("t_nbsb", [NT, steps], f32))
        lastbuf = ctx.enter_context(nc.sbuf_tensor("t_lastbuf", [NT, steps], f32))
        dA = ctx.enter_context(nc.sbuf_tensor("t_dA", [NT, W], f32))
        dB = ctx.enter_context(nc.sbuf_tensor("t_dB", [NT, W], f32))
        candT = [ctx.enter_context(nc.sbuf_tensor(f"t_candT{k}", [NT, NT], f32))
                 for k in range(4)]
        psall = ctx.enter_context(nc.psum_tensor("t_psall", [NT, 2048], f32))
        psb = [psall[:, k * 512:k * 512 + NT] for k in range(4)]
        fb = [ctx.enter_context(nc.sbuf_tensor(f"t_fb{k}", [NT, 1], f32)) for k in range(4)]
        fbb = [ctx.enter_context(nc.sbuf_tensor(f"t_fbb{k}", [NT, 1], f32)) for k in range(4)]
        p0 = ctx.enter_context(nc.sbuf_tensor("t_p0", [NT, W], f32))
        dsem = ctx.enter_context(nc.semaphore("dsem"))
        vs = ctx.enter_context(nc.semaphore("vs"))
        asem = ctx.enter_context(nc.semaphore("asem"))
        psem = ctx.enter_context(nc.semaphore("psem"))
        block = ctx.enter_context(nc.Block())

        delta = [dA, dB]

        @block.gpsimd
        def _(g):
            for dst, src in ((trans, trans_d), (mask, mask_d), (ident, ident_d),
                             (dA, d0_d), (bsb, b_d), (nbsb, nb_d)):
                g.dma_start(dst[:], src[:]).then_inc(dsem, 16)
            g.wait_ge(asem, 3 * steps)
            g.wait_ge(vs, 1 + 4 * steps)
            g.dma_start(last_o[:], lastbuf[:]).then_inc(dsem, 16)
            g.dma_start(df_o[:], delta[steps % 2][:]).then_inc(dsem, 16)

        @block.vector
        def _(v):
            v.wait_ge(dsem, 6 * 16)
            v.memset(dB[:], -1e9).then_inc(vs, 1)
            for i in range(steps):
                cur, nxt = delta[i % 2], delta[(i + 1) % 2]
                j = i % 4
                v.tensor_scalar_add(candT[j][:], trans[:],
                                    cur[:, W - 1:W]).then_inc(vs, 1)
                v.wait_ge(psem, i + 1)
                v.reduce_max(fb[j][:], psb[j], axis=AX).then_inc(vs, 1)
                v.wait_ge(asem, 3 * i + 3)
                v.tensor_scalar_add(p0[:], mask[:], fbb[j][:]).then_inc(vs, 1)
                v.tensor_max(nxt[:, 1:W], nxt[:, 1:W],
                             p0[:, 1:W]).then_inc(vs, 1)

        @block.scalar
        def _(a):
            a.wait_ge(dsem, 6 * 16)
            for i in range(steps):
                cur = delta[i % 2]
                j = i % 4
                a.wait_ge(vs, 1 + 4 * i)
                a.activation(lastbuf[:, i:i + 1], cur[:, W - 1:W],
                             IDENT).then_inc(asem, 1)
                a.activation(delta[(i + 1) % 2][:, 1:W], cur[:, 0:W - 1], IDENT,
                             bias=nbsb[:, i:i + 1]).then_inc(asem, 1)
                a.wait_ge(vs, 4 * i + 3)
                a.activation(fbb[j][:], fb[j][:], IDENT,
                             bias=bsb[:, i:i + 1]).then_inc(asem, 1)

        @block.tensor
        def _(p):
            p.wait_ge(dsem, 6 * 16)
            for i in range(steps):
                j = i % 4
                p.wait_ge(vs, 4 * i + 2)
                p.transpose(psb[j], candT[j][:],
                            ident[:]).then_inc(psem, 1)

    _NC_CACHE[steps] = nc
    return nc


def _forward_numpy(b_lp, nb_lp, trans):
    """Fallback: full-S Viterbi forward in numpy; returns lastout, delta_final."""
    steps = b_lp.shape[0] - 1
    delta = np.where(POS == 0, b_lp[0], nb_lp[0]).astype(np.float32) - np.float32(
        np.log(np.float32(S)))
    lastout = np.zeros((NT, steps), np.float32)
    for i in range(steps):
        prev_last = delta[LAST_IDX]
        lastout[:, i] = prev_last
        fb = (prev_last[:, None] + trans).max(0)
        shifted = np.empty_like(delta)
        shifted[1:] = delta[:-1]
        shifted[0] = NEG
        shifted[FIRST_IDX] = fb
        delta = shifted + np.where(POS == 0, b_lp[i + 1], nb_lp[i + 1]).astype(
            np.float32)
    dfin = np.full((NT, W), NEG, np.float32)
    dfin[ROWMAP, COLMAP] = delta
    return lastout, dfin


def kernel(logit):
    logit = np.asarray(logit, dtype=np.float32)
    B, T = logit.shape
    steps = T - 1
    trans = _trans_log()
    b_lp = _log_sigmoid(logit)
    nb_lp = _log_sigmoid(-logit)

    results = None
    try:
        nc = _build(steps)
        mask = np.full((NT, W), NEG, np.float32)
        mask[np.arange(NT), W - INTERVALS] = 0.0
        ident = np.eye(NT, dtype=np.float32)
        logS = np.float32(np.log(np.float32(S)))
        in_maps = []
        for b in range(B):
            d0 = np.full((NT, W), NEG, np.float32)
            for ti, tau in enumerate(INTERVALS):
                d0[ti, W - tau:] = nb_lp[b, 0] - logS
                d0[ti, W - tau] = b_lp[b, 0] - logS
            in_maps.append({
                "trans": trans, "mask": mask, "ident": ident, "delta0": d0,
                "beat": np.ascontiguousarray(
                    np.broadcast_to(b_lp[b, 1:], (NT, steps))),
                "nonbeat": np.ascontiguousarray(
                    np.broadcast_to(nb_lp[b, 1:], (NT, steps))),
            })
        from concourse.bass_utils import run_bass_kernel_spmd
        global LAST_RESULTS
        LAST_RESULTS = run_bass_kernel_spmd(nc, in_maps, core_ids=list(range(B)))
        results = LAST_RESULTS.results
    except Exception as e:
        print(f"kernel: device path failed ({e!r}); numpy fallback", file=sys.stderr)

    act = 1.0 / (1.0 + np.exp(-logit.astype(np.float64)))
    out = np.zeros((B, T), np.float32)
    for b in range(B):
        if results is not None:
            lastout = results[b]["lastout"]
            dfin = results[b]["deltaout"]
        else:
            lastout, dfin = _forward_numpy(b_lp[b], nb_lp[b], trans)
        vals = dfin[ROWMAP, COLMAP]
        s = int(np.argmax(vals))
        onb = np.zeros(T, bool)
        for i in range(steps - 1, -1, -1):
            p = POS[s]
            onb[i + 1] = p == 0
            if p == 0:
                cand = lastout[:, i] + trans[:, ROWMAP[s]]
                s = int(LAST_IDX[int(np.argmax(cand))])
            else:
                s -= 1
        onb[0] = POS[s] == 0
        out[b] = (onb & (act[b] >= 0.05)).astype(np.float32)
    return out

